# revision 3
# baseline (speedup 1.0000x reference)
"""HGNN message-passing kernel for 8 Trainium2 NeuronCores.

Strategy (node-sharded, per spec sharding_hint):
- 6250 nodes per core; X kept on-chip feature-major (X^T, two [128, 6272] tiles).
- Linear layers computed locally per core (weights replicated, fp32r matmuls).
- node2edge: per-core nnz (those whose node is local) sorted by dest edge;
  gather a-scaled Xf rows from a per-core DRAM table via dma_gather; segment-sum
  via one-hot S matmuls into PSUM per 128-edge window; per-core partial E
  b-scaled and AllReduce-summed across the 8 cores.
- edge2node: per-core nnz sorted by dest node; gather E rows from the
  AllReduced table; rows get b*leaky via ScalarE Prelu; S carries the a-scale;
  skip connection + bias accumulate in the same PSUM window.
- coeff = d_V[n]^-0.5 * d_E[e]^-1 is separable (a[n]*b[e]); a is folded into
  row production / S, b into window eviction / row scaling. leaky commutes with
  positive scales, which keeps everything exact.
"""
import sys
sys.path.insert(0, "/opt/trn_rl_repo")
import numpy as np

N_NODES, N_EDGES, NNZ, D, L = 50000, 10000, 400000, 256, 2
C = 8                      # cores
NPC = N_NODES // C         # 6250 nodes per core
NW = 49                    # node windows of 128 (6272)
NPC_PAD = NW * 128
EW = 79                    # edge windows of 128 (10112)
E_PAD = EW * 128
GATH = 2048                # indices per dma_gather piece (16 chunks)
GCH = GATH // 128          # chunks per gather piece
NEG_SLOPE = 0.1
NQ = 4                     # SWDGE queues

_cache = {}


def _wrap_idxs_piece(piece):
    # [2048] -> int16 [128, 128]: index j at partition j%16 (replicated over
    # the 8 Q7 core groups), column j//16
    w = piece.reshape(GATH // 16, 16).T.astype(np.int16)
    return np.tile(w, (8, 1))


def _build_chunks(order_src, order_off, order_win, n_win, cpw, extras=()):
    """Pack destination-sorted nnz into 128-row chunks per window.

    order_*: arrays sorted by destination. cpw: chunks per window (shared
    across cores). extras: list of (values_sorted, pad_value).
    Returns idx [T*128] int64, dst [T*128] f32 (-1 padding), extras packed."""
    T = int(sum(cpw))
    idx = np.zeros(T * 128, np.int64)
    dst = np.full(T * 128, -1.0, np.float32)
    outs = [np.full(T * 128, pv, np.float32) for _, pv in extras]
    counts = np.bincount(order_win, minlength=n_win)
    pos = 0
    base = 0
    for w in range(n_win):
        cnt = int(counts[w])
        cap = int(cpw[w]) * 128
        assert cnt <= cap
        idx[base:base + cnt] = order_src[pos:pos + cnt]
        dst[base:base + cnt] = order_off[pos:pos + cnt]
        for oi, (vals, _) in enumerate(extras):
            outs[oi][base:base + cnt] = vals[pos:pos + cnt]
        pos += cnt
        base += cap
    return idx, dst, outs


def _pack_cols(arr_flat, T):
    # [T*128] -> [128, T] (partition = within-chunk slot, column = chunk)
    return np.ascontiguousarray(arr_flat.reshape(T, 128).T)


def _pack_idx(idx_flat):
    # [T*128] -> wrapped int16 [128, npieces*128], padding with 0
    n = idx_flat.shape[0]
    npieces = -(-n // GATH)
    padded = np.zeros(npieces * GATH, np.int64)
    padded[:n] = idx_flat
    return np.concatenate(
        [_wrap_idxs_piece(padded[g * GATH:(g + 1) * GATH]) for g in range(npieces)],
        axis=1), npieces


def _preprocess(X, node_ids, edge_ids, weights):
    node_ids = np.asarray(node_ids).astype(np.int64)
    edge_ids = np.asarray(edge_ids).astype(np.int64)
    X = np.asarray(X, np.float32)

    deg_v = np.bincount(node_ids, minlength=N_NODES).astype(np.float32)
    deg_e = np.bincount(edge_ids, minlength=N_EDGES).astype(np.float32)
    a = deg_v ** -0.5
    b = 1.0 / deg_e

    core_of = node_ids // NPC
    percore = []
    n2e_counts = np.zeros((C, EW), np.int64)
    e2n_counts = np.zeros((C, NW), np.int64)
    sorted_data = []
    for c in range(C):
        sel = core_of == c
        loc_node = node_ids[sel] - c * NPC
        edge = edge_ids[sel]
        o1 = np.argsort(edge, kind="stable")
        n2e_counts[c] = np.bincount(edge[o1] // 128, minlength=EW)
        o2 = np.argsort(loc_node, kind="stable")
        e2n_counts[c] = np.bincount(loc_node[o2] // 128, minlength=NW)
        sorted_data.append(((loc_node[o1], edge[o1]), (edge[o2], loc_node[o2])))

    cpw_n2e = np.maximum(1, -(-n2e_counts.max(axis=0) // 128))   # [EW]
    cpw_e2n = np.maximum(1, -(-e2n_counts.max(axis=0) // 128))   # [NW]
    T_n2e = int(cpw_n2e.sum())
    T_e2n = int(cpw_e2n.sum())

    b_pad = np.zeros(E_PAD, np.float32)
    b_pad[:N_EDGES] = b
    b_win = np.ascontiguousarray(b_pad.reshape(EW, 128).T)       # [128, EW]
    iota = np.tile(np.arange(128, dtype=np.float32)[None, :], (128, 1))
    ones = np.ones((1, 128), np.float32)

    npc1 = npc2 = None
    for c in range(C):
        (n2e_src, n2e_edge), (e2n_edge, e2n_node) = sorted_data[c]
        idx1, dst1, _ = _build_chunks(
            n2e_src, (n2e_edge % 128).astype(np.float32), n2e_edge // 128,
            EW, cpw_n2e)
        idx1_w, npc1 = _pack_idx(idx1)
        dst1_p = _pack_cols(dst1, T_n2e)

        a_vals = a[e2n_node + c * NPC].astype(np.float32)
        b_vals = b[e2n_edge].astype(np.float32)
        idx2, dst2, (a2, b2) = _build_chunks(
            e2n_edge, (e2n_node % 128).astype(np.float32), e2n_node // 128,
            NW, cpw_e2n, extras=((a_vals, 0.0), (b_vals, 1.0)))
        idx2_w, npc2 = _pack_idx(idx2)
        dst2_p = _pack_cols(dst2, T_e2n)
        a2_p = _pack_cols(a2, T_e2n)
        b2_p = _pack_cols(b2, T_e2n)

        xcT = np.zeros((D, NPC_PAD), np.float32)
        xcT[:, :NPC] = X[c * NPC:(c + 1) * NPC].T
        a_c = np.zeros(NPC_PAD, np.float32)
        a_c[:NPC] = a[c * NPC:(c + 1) * NPC]
        a_win = np.ascontiguousarray(a_c.reshape(NW, 128).T)     # [128, NW]

        m = {
            "xcT": xcT, "a_win": a_win, "b_win": b_win, "iota": iota,
            "ones": ones,
            "n2e_idx": idx1_w, "n2e_dst": dst1_p,
            "e2n_idx": idx2_w, "e2n_dst": dst2_p,
            "e2n_a": a2_p, "e2n_b": b2_p,
        }
        for hl in range(4):
            fcw, fcb, pjw, pjb = weights[hl]
            m[f"fcwT{hl}"] = np.ascontiguousarray(fcw.T)
            m[f"fcb{hl}"] = np.ascontiguousarray(fcb[None, :])
            m[f"pjwT{hl}"] = np.ascontiguousarray(pjw.T)
            m[f"pjb{hl}"] = np.ascontiguousarray(pjb[None, :])
        percore.append(m)

    meta = {
        "cpw_n2e": [int(v) for v in cpw_n2e],
        "cpw_e2n": [int(v) for v in cpw_e2n],
        "npieces_n2e": npc1, "npieces_e2n": npc2,
    }
    return percore, meta


def _build_program(meta):
    from contextlib import ExitStack
    from concourse import bass, mybir, bacc, tile
    from concourse.masks import make_identity

    F32 = mybir.dt.float32
    F32R = mybir.dt.float32r
    I16 = mybir.dt.int16
    AF = mybir.ActivationFunctionType
    cpw_n2e = meta["cpw_n2e"]
    cpw_e2n = meta["cpw_e2n"]
    T_n2e = sum(cpw_n2e)
    T_e2n = sum(cpw_e2n)
    np1 = meta["npieces_n2e"]
    np2 = meta["npieces_e2n"]

    nc = bacc.Bacc("TRN2", target_bir_lowering=False, debug=False,
                   num_devices=C, num_swdge_queues=NQ)

    t_xcT = nc.dram_tensor("xcT", [D, NPC_PAD], F32R, kind="ExternalInput")
    t_awin = nc.dram_tensor("a_win", [128, NW], F32, kind="ExternalInput")
    t_bwin = nc.dram_tensor("b_win", [128, EW], F32, kind="ExternalInput")
    t_iota = nc.dram_tensor("iota", [128, 128], F32, kind="ExternalInput")
    t_ones = nc.dram_tensor("ones", [1, 128], F32R, kind="ExternalInput")
    t_n2e_idx = nc.dram_tensor("n2e_idx", [128, np1 * 128], I16, kind="ExternalInput")
    t_n2e_dst = nc.dram_tensor("n2e_dst", [128, T_n2e], F32, kind="ExternalInput")
    t_e2n_idx = nc.dram_tensor("e2n_idx", [128, np2 * 128], I16, kind="ExternalInput")
    t_e2n_dst = nc.dram_tensor("e2n_dst", [128, T_e2n], F32, kind="ExternalInput")
    t_e2n_a = nc.dram_tensor("e2n_a", [128, T_e2n], F32, kind="ExternalInput")
    t_e2n_b = nc.dram_tensor("e2n_b", [128, T_e2n], F32, kind="ExternalInput")
    t_w = {}
    for hl in range(4):
        t_w[hl] = (
            nc.dram_tensor(f"fcwT{hl}", [D, D], F32R, kind="ExternalInput"),
            nc.dram_tensor(f"fcb{hl}", [1, D], F32R, kind="ExternalInput"),
            nc.dram_tensor(f"pjwT{hl}", [D, D], F32R, kind="ExternalInput"),
            nc.dram_tensor(f"pjb{hl}", [1, D], F32R, kind="ExternalInput"),
        )
    t_Efin = nc.dram_tensor("E_final", [N_EDGES, D], F32, kind="ExternalOutput")
    t_Xout = nc.dram_tensor("X_out", [NPC, D], F32, kind="ExternalOutput")

    with tile.TileContext(nc) as tc:
        with ExitStack() as ctx:
            const = ctx.enter_context(tc.tile_pool(name="const", bufs=1))
            wpool = ctx.enter_context(tc.tile_pool(name="w", bufs=1))
            gpool = ctx.enter_context(tc.tile_pool(name="g", bufs=3))
            spool = ctx.enter_context(tc.tile_pool(name="s", bufs=6))
            epool = ctx.enter_context(tc.tile_pool(name="e", bufs=4))
            pspool = ctx.enter_context(tc.tile_pool(name="ps", bufs=4, space="PSUM"))
            ptpool = ctx.enter_context(tc.tile_pool(name="pt", bufs=2, space="PSUM"))
            dram = ctx.enter_context(tc.tile_pool(name="dram", bufs=1, space="DRAM"))

            # ---- persistent loads
            xcT = [const.tile([128, NPC_PAD], F32R, tag=f"xcT{h}", name=f"xcT{h}") for h in range(2)]
            for h in range(2):
                nc.sync.dma_start(out=xcT[h][:], in_=t_xcT[h * 128:(h + 1) * 128, :])
            a_win = const.tile([128, NW], F32)
            b_win = const.tile([128, EW], F32)
            iota = const.tile([128, 128], F32)
            ones = const.tile([1, 128], F32R)
            nc.sync.dma_start(out=a_win[:], in_=t_awin[:])
            nc.sync.dma_start(out=b_win[:], in_=t_bwin[:])
            nc.sync.dma_start(out=iota[:], in_=t_iota[:])
            nc.sync.dma_start(out=ones[:], in_=t_ones[:])
            ident = const.tile([128, 128], F32)
            make_identity(nc, ident[:])
            # index/metadata tables (reused by every op) loaded once
            bidx = const.tile([128, np1 * 128], I16)
            bdst = const.tile([128, T_n2e], F32)
            cidx = const.tile([128, np2 * 128], I16)
            cdst = const.tile([128, T_e2n], F32)
            ca = const.tile([128, T_e2n], F32)
            cb = const.tile([128, T_e2n], F32)
            for tl, src in ((bidx, t_n2e_idx), (bdst, t_n2e_dst),
                            (cidx, t_e2n_idx), (cdst, t_e2n_dst),
                            (ca, t_e2n_a), (cb, t_e2n_b)):
                nc.sync.dma_start(out=tl[:], in_=src[:])
            W = {}
            for hl in range(4):
                fcw, fcb, pjw, pjb = t_w[hl]
                fw = [wpool.tile([128, D], F32R, tag=f"fw{hl}{h}", name=f"fw{hl}{h}") for h in range(2)]
                pw = [wpool.tile([128, D], F32R, tag=f"pw{hl}{h}", name=f"pw{hl}{h}") for h in range(2)]
                for h in range(2):
                    nc.sync.dma_start(out=fw[h][:], in_=fcw[h * 128:(h + 1) * 128, :])
                    nc.sync.dma_start(out=pw[h][:], in_=pjw[h * 128:(h + 1) * 128, :])
                fb = wpool.tile([1, D], F32R, tag=f"fb{hl}")
                pb = wpool.tile([1, D], F32R, tag=f"pb{hl}")
                nc.sync.dma_start(out=fb[:], in_=fcb[:])
                nc.sync.dma_start(out=pb[:], in_=pjb[:])
                W[hl] = (fw, fb, pw, pb)

            XfP = dram.tile([NPC_PAD, D], F32R, tag="XfP")
            E_part = dram.tile([E_PAD, D], F32, tag="Epart")
            E_sum = dram.tile([E_PAD, D], F32, tag="Esum")

            def phase_A(hl):
                """XfP <- a * (X @ fcW.T + fcB), row-major per node window."""
                fw, fb, _, _ = W[hl]
                for w in range(NW):
                    sl = slice(w * 128, (w + 1) * 128)
                    ps = pspool.tile([128, D], F32, tag="ps")
                    nc.tensor.matmul(ps[:], xcT[0][:, sl], fw[0][:], start=True, stop=False)
                    nc.tensor.matmul(ps[:], xcT[1][:, sl], fw[1][:], start=False, stop=False)
                    nc.tensor.matmul(ps[:], ones[:1, :], fb[:1, :], start=False, stop=True)
                    ev = epool.tile([128, D], F32R, tag="xf")
                    nc.scalar.activation(ev[:], ps[:], AF.Copy, scale=a_win[:, w:w + 1])
                    nc.sync.dma_start(out=XfP[sl, :], in_=ev[:])

            def phase_B():
                """E_part <- b * segsum(gather(XfP)), then AllReduce -> E_sum."""
                gts = {}

                def get_piece(g):
                    if g not in gts:
                        gt = gpool.tile([128, GCH, D], F32R, tag="g")
                        nc.gpsimd.dma_gather(
                            gt[:], XfP[:], bidx[:, g * 128:(g + 1) * 128],
                            GATH, GATH, D, single_packet=False, queue_num=g % NQ)
                        gts[g] = gt
                    return gts[g]

                ci = 0
                for w in range(EW):
                    ps = pspool.tile([128, D], F32, tag="ps")
                    nch = cpw_n2e[w]
                    for j in range(nch):
                        gt = get_piece(ci // GCH)
                        S = spool.tile([128, 128], F32R, tag="S")
                        nc.vector.tensor_scalar(
                            S[:], iota[:], bdst[:, ci:ci + 1], None,
                            op0=mybir.AluOpType.is_equal)
                        nc.tensor.matmul(ps[:], S[:], gt[:, ci % GCH, :],
                                         start=(j == 0), stop=(j == nch - 1))
                        ci += 1
                    ev = epool.tile([128, D], F32, tag="ee")
                    nc.scalar.activation(ev[:], ps[:], AF.Copy, scale=b_win[:, w:w + 1])
                    nc.sync.dma_start(out=E_part[w * 128:(w + 1) * 128, :], in_=ev[:])
                nc.gpsimd.collective_compute(
                    "AllReduce", mybir.AluOpType.add,
                    replica_groups=[list(range(C))],
                    ins=[E_part[:].opt()], outs=[E_sum[:].opt()])

            def phase_C(hl, last):
                """X <- leaky(a*segsum(b*leaky(gather(E_sum))) + X@pjW.T+pjB)."""
                _, _, pw, pb = W[hl]
                gts = {}

                def get_piece(g):
                    if g not in gts:
                        gt = gpool.tile([128, GCH, D], F32R, tag="g")
                        nc.gpsimd.dma_gather(
                            gt[:], E_sum[:].bitcast(F32R),
                            cidx[:, g * 128:(g + 1) * 128],
                            GATH, GATH, D, single_packet=False, queue_num=g % NQ)
                        gts[g] = gt
                    return gts[g]

                ci = 0
                for w in range(NW):
                    sl = slice(w * 128, (w + 1) * 128)
                    ps = pspool.tile([128, D], F32, tag="ps")
                    nc.tensor.matmul(ps[:], xcT[0][:, sl], pw[0][:], start=True, stop=False)
                    nc.tensor.matmul(ps[:], xcT[1][:, sl], pw[1][:], start=False, stop=False)
                    nc.tensor.matmul(ps[:], ones[:1, :], pb[:1, :], start=False, stop=False)
                    ncch = cpw_e2n[w]
                    for j in range(ncch):
                        gt = get_piece(ci // GCH)
                        rt = epool.tile([128, D], F32R, tag="rt")
                        nc.scalar.activation(rt[:], gt[:, ci % GCH, :], AF.Prelu,
                                             scale=cb[:, ci:ci + 1], alpha=NEG_SLOPE)
                        S = spool.tile([128, 128], F32R, tag="S")
                        nc.vector.tensor_scalar(
                            S[:], iota[:], cdst[:, ci:ci + 1], ca[:, ci:ci + 1],
                            op0=mybir.AluOpType.is_equal, op1=mybir.AluOpType.mult)
                        nc.tensor.matmul(ps[:], S[:], rt[:],
                                         start=False, stop=(j == ncch - 1))
                        ci += 1
                    xw = epool.tile([128, D], F32, tag="xw")
                    nc.scalar.activation(xw[:], ps[:], AF.Prelu, alpha=NEG_SLOPE)
                    if not last:
                        for h in range(2):
                            pt = ptpool.tile([128, 128], F32, tag="pt")
                            nc.tensor.transpose(pt[:], xw[:, h * 128:(h + 1) * 128],
                                                ident[:])
                            nc.scalar.activation(xcT[h][:, sl], pt[:], AF.Copy)
                    else:
                        nrows = 128 if w < NW - 1 else NPC - (NW - 1) * 128
                        nc.sync.dma_start(out=t_Xout[w * 128:w * 128 + nrows, :],
                                          in_=xw[:nrows, :])
                        xa = epool.tile([128, D], F32R, tag="xa")
                        nc.scalar.activation(xa[:], ps[:], AF.Prelu,
                                             scale=a_win[:, w:w + 1], alpha=NEG_SLOPE)
                        nc.sync.dma_start(out=XfP[sl, :], in_=xa[:])

            for hl in range(4):
                phase_A(hl)
                phase_B()
                phase_C(hl, last=(hl == 3))
            phase_B()   # final node2edge on a*X
            for w in range(EW):
                lo = w * 128
                hi = min((w + 1) * 128, N_EDGES)
                et = epool.tile([128, D], F32, tag="ecp")
                nc.sync.dma_start(out=et[:hi - lo, :], in_=E_sum[lo:hi, :])
                nc.sync.dma_start(out=t_Efin[lo:hi, :], in_=et[:hi - lo, :])

    nc.compile()
    return nc


def kernel(X, node_ids, edge_ids, n_hyperedges,
           fc1_w, fc1_b, proj1_w, proj1_b, fc2_w, fc2_b, proj2_w, proj2_b):
    weights = []
    for l in range(L):
        weights.append((np.asarray(fc1_w[l], np.float32), np.asarray(fc1_b[l], np.float32),
                        np.asarray(proj1_w[l], np.float32), np.asarray(proj1_b[l], np.float32)))
        weights.append((np.asarray(fc2_w[l], np.float32), np.asarray(fc2_b[l], np.float32),
                        np.asarray(proj2_w[l], np.float32), np.asarray(proj2_b[l], np.float32)))
    percore, meta = _preprocess(X, node_ids, edge_ids, weights)

    key = (tuple(meta["cpw_n2e"]), tuple(meta["cpw_e2n"]))
    if key not in _cache:
        _cache[key] = _build_program(meta)
    nc = _cache[key]

    from concourse.bass_utils import run_bass_kernel_spmd
    res = run_bass_kernel_spmd(nc, percore, core_ids=list(range(C)))
    E_final = res.results[0]["E_final"]
    X_out = np.concatenate([res.results[c]["X_out"] for c in range(C)], axis=0)
    return (E_final, X_out)
